# revision 8
# baseline (speedup 1.0000x reference)
"""MoE layer (top-2 of 8 experts, D=1024, H=2048) on 8 trn2 NeuronCores.

Strategy (expert-parallel, per the sharding hint):
  - Router (tiny: [16384,1024]@[1024,8]) runs on host; its output IS the
    sharding decision ("all-to-all tokens by expert assignment").
  - Core e receives the tokens routed to expert e, sorted by gate weight
    descending and padded to a uniform capacity, plus expert e's weights:
      * the 3584 highest-gate tokens run in bf16 (7 column blocks of 512),
      * the next 512 (lowest-gate on device) run in fp8 e4m3 with
        DoubleRow matmuls (2 fp8 MACs/cell/cycle) — their small gate
        weight bounds the extra quantization error,
      * any overflow beyond 4096 is computed exactly on host (f32).
    Device math per block:
        stage 1: hT[h, c] = relu(sum_d w1[d,h] * xT[d,c] (+ b1[h]))
        stage 2: yT[d, c] = sum_h w2[h,d] * hT[h,c]
    with f32 PSUM accumulation; y streams out in bf16.
  - Host scatter-adds gate * (y + b2) into the output; fp8 scale factors
    are folded into the per-token gates.
"""

import numpy as np
import ml_dtypes

import concourse.bacc as bacc
import concourse.mybir as mybir
import concourse.tile as tile
from concourse.tile_rust import add_dep_helper
from concourse import bass_utils

B, S, D, E, TOPK = 4, 4096, 1024, 8, 2
H = 2 * D
P = 128
KD = D // P    # 8 k-tiles over D
MH = H // P    # 16 h-tiles over H
ND = D // P    # 8 d-tiles over D
FD = 512       # moving free-dim per matmul / column block
N_CORES = 8

# fp8 scaling: s_x * s_w1 == 2**15 (so the compiled relu scale 2**-10
# yields h_f8 = 32*relu(x@w1)), s_h = 32, s_w2 = 1024. Host divides the
# fp8 tokens' gates by s_h*s_w2 = 2**15.
S_X = 32.0
S_W1 = 1024.0
S_H = 32.0
S_W2 = 1024.0
F8_CLIP = 240.0  # TRN FP8_EXP4 max normal

BF16 = mybir.dt.bfloat16
F32 = mybir.dt.float32
F8 = mybir.dt.float8e4
NP_BF16 = ml_dtypes.bfloat16
NP_F8 = ml_dtypes.float8_e4m3

N_F8_BLOCKS = 1  # 0 = all-bf16; 1 = last 512 device tokens in fp8

_cache = {}           # (C_bf, n_f8, with_b1) -> compiled Bacc
last_run_results = None  # BassKernelResults of the most recent device run


def _build(C_bf, n_f8=N_F8_BLOCKS, with_b1=True):
    """Build + compile the per-core FFN program.

    C_bf bf16 token columns (multiple of 512) + n_f8*512 fp8 columns.
    Weight dram layouts are tile-major (host pre-transposes):
      w1m[m, p, k, j] = w1[k*128+p, m*128+j]   (m-major: stage-1 weight
        column-tile m is a contiguous 256KB chunk -> the first matmul
        only needs w1m[0] + the first x k-slice, so compute starts ~3us
        into the kernel instead of after the full weight load)
      w2d[d, p, m, j] = w2[m*128+p, d*128+j]   (d-major, same idea)
      w1q/w2q: same layout, fp8, pre-scaled by S_W1/S_W2.
    """
    assert C_bf % FD == 0
    C_f8 = n_f8 * FD
    C = C_bf + C_f8
    nc = bacc.Bacc("TRN2", target_bir_lowering=False, debug=False)
    xT = nc.dram_tensor("xT", [D, C_bf], BF16, kind="ExternalInput").ap()
    w1m = nc.dram_tensor("w1m", [MH, P, KD, P], BF16, kind="ExternalInput").ap()
    w2d = nc.dram_tensor("w2d", [ND, P, MH, P], BF16, kind="ExternalInput").ap()
    if n_f8:
        xq = nc.dram_tensor("xq", [D, C_f8], F8, kind="ExternalInput").ap()
        w1qd = nc.dram_tensor("w1q", [MH, P, KD, P], F8, kind="ExternalInput").ap()
        w2qd = nc.dram_tensor("w2q", [ND, P, MH, P], F8, kind="ExternalInput").ap()
    b1t = (
        nc.dram_tensor("b1t", [P, MH], F32, kind="ExternalInput").ap()
        if with_b1 else None
    )
    yT = nc.dram_tensor("yT", [D, C], BF16, kind="ExternalOutput").ap()

    nbf = C_bf // FD
    blocks = [("bf", i * FD) for i in range(nbf)] + [
        ("f8", C_bf + i * FD) for i in range(n_f8)
    ]
    nblk = len(blocks)

    xT_r = xT.rearrange("(k p) c -> p k c", p=P)      # [P, KD, C_bf]
    yT_r = yT.rearrange("(d p) c -> p d c", p=P)      # [P, ND, C]
    w1m_r = w1m.rearrange("m p k j -> p m (k j)")     # [P, MH, KD*P]
    w2d_r = w2d.rearrange("d p m j -> p d (m j)")     # [P, ND, MH*P]
    if n_f8:
        xq_r = xq.rearrange("(k p) c -> p k c", p=P)  # [P, KD, C_f8]
        w1q_r = w1qd.rearrange("m p k j -> p m k j")  # [P, MH, KD, P]
        w2q_r = w2qd.rearrange("d p m j -> p d m j")  # [P, ND, MH, P]

    with tile.TileContext(nc) as tc:
        with (
            tc.tile_pool(name="wpool", bufs=1) as wpool,
            tc.tile_pool(name="xpool", bufs=3) as xpool,
            tc.tile_pool(name="hpool", bufs=2) as hpool,
            tc.tile_pool(name="ypool", bufs=2) as ypool,
            tc.tile_pool(name="ps1", bufs=4, space="PSUM") as ps1,
            tc.tile_pool(name="ps2", bufs=4, space="PSUM") as ps2,
        ):
            # --- startup: interleave the first x block's k-slices with the
            # first w1 column-tiles so the first matmul chain starts as soon
            # as (x k0, w1 m0) land, and is paced by the DMA pipe after.
            xb0 = xpool.tile([P, KD, FD], BF16)
            w1_sb = wpool.tile([P, MH, KD * P], BF16)
            # The HWDGE round-robins sync/scalar triggers (sync first), so
            # splitting the startup DMAs across the two rings in alternating
            # program order realizes this exact global transfer order:
            #   w1[m0,k0] -> x[k0] -> w1[m0,k1-7] -> x[k1-3] -> x[k4-7]
            #   -> w1[m1] -> w1[m2-3] -> w1[m4-7]
            # i.e. the first matmul chain starts after ~450KB, never
            # starving either operand.
            nc.sync.dma_start(w1_sb[:, 0, :P], w1m_r[:, 0, :P])
            nc.scalar.dma_start(xb0[:, 0, :], xT_r[:, 0, 0:FD])
            nc.sync.dma_start(w1_sb[:, 0, P:], w1m_r[:, 0, P:])
            nc.scalar.dma_start(xb0[:, 1:4, :], xT_r[:, 1:4, 0:FD])
            nc.sync.dma_start(xb0[:, 4:8, :], xT_r[:, 4:8, 0:FD])
            nc.scalar.dma_start(w1_sb[:, 1, :], w1m_r[:, 1, :])
            nc.sync.dma_start(w1_sb[:, 2:4, :], w1m_r[:, 2:4, :])
            nc.scalar.dma_start(w1_sb[:, 4:8, :], w1m_r[:, 4:8, :])
            if with_b1:
                b1_sb = wpool.tile([P, MH], F32)
                nc.gpsimd.dma_start(b1_sb[:], b1t[:, :])

            # Bulk weights stream on the gpsimd SWDGE path, gated on the
            # first x block having landed so they can't starve it of HBM
            # bandwidth: a tiny DVE copy reading xb0's last k-slice gives
            # every bulk DMA a dep edge on that DMA.
            gate_sb = wpool.tile([P, 8], BF16)
            gate_copy = nc.vector.tensor_copy(gate_sb[:1, :8], xb0[:1, 7, :8])
            w2_sb = wpool.tile([P, ND, MH * P], BF16)

            # One ordered input stream on the SWDGE ring, sequenced by the
            # time each chunk is first needed, so a later x block can never
            # jump the queue ahead of stage-1/2 weights on the DMA pipe.
            bulk = []
            xbs = [xb0]

            def _xdma():
                i = len(xbs)
                xb = xpool.tile([P, KD, FD], BF16)
                xbs.append(xb)
                bulk.append(nc.gpsimd.dma_start(
                    xb[:, :, :], xT_r[:, :, i * FD:(i + 1) * FD]
                ))

            bulk.append(nc.gpsimd.dma_start(
                w1_sb[:, 8:12, :], w1m_r[:, 8:12, :]
            ))
            if nbf > 1:
                _xdma()
            bulk.append(nc.gpsimd.dma_start(
                w1_sb[:, 12:16, :], w1m_r[:, 12:16, :]
            ))
            bulk.append(nc.gpsimd.dma_start(
                w2_sb[:, 0:4, :], w2d_r[:, 0:4, :]
            ))
            if nbf > 2:
                _xdma()
            bulk.append(nc.gpsimd.dma_start(
                w2_sb[:, 4:8, :], w2d_r[:, 4:8, :]
            ))
            if nbf > 3:
                _xdma()
            if n_f8:
                w1q_sb = wpool.tile([P, MH, KD, P], F8)
                w2q_sb = wpool.tile([P, ND, MH, P], F8)
                xq_sb = wpool.tile([P, KD, C_f8], F8)
                bulk.append(nc.gpsimd.dma_start(w1q_sb[:], w1q_r[:]))
                bulk.append(nc.gpsimd.dma_start(w2q_sb[:], w2q_r[:]))
                bulk.append(nc.gpsimd.dma_start(xq_sb[:], xq_r[:]))
            while len(xbs) < nbf:
                _xdma()
            for w in bulk:
                add_dep_helper(
                    w.ins, gate_copy.ins,
                    reason="bulk input stream waits for x block 0 to land",
                )

            live = {}

            def stage1(blki):
                kind, c0 = blocks[blki]
                hT = hpool.tile([P, MH, FD], BF16 if kind == "bf" else F8)
                if kind == "f8":
                    live[blki] = (None, hT)
                    for m in range(MH):
                        ps = ps1.tile([P, FD], F32)
                        for kp in range(KD // 2):
                            nc.tensor.matmul(
                                ps[:, :],
                                w1q_sb[:, m, 2 * kp:2 * kp + 2, :],
                                xq_sb[:, 2 * kp:2 * kp + 2, c0 - C_bf:c0 - C_bf + FD],
                                start=(kp == 0),
                                stop=(kp == KD // 2 - 1),
                                perf_mode=mybir.MatmulPerfMode.DoubleRow,
                            )
                        nc.scalar.activation(
                            hT[:, m, :],
                            ps[:, :],
                            mybir.ActivationFunctionType.Relu,
                            bias=b1q_sb[:, m:m + 1] if with_b1 else 0.0,
                            scale=S_H / (S_X * S_W1),
                        )
                    return
                xb = xbs[blki]
                live[blki] = (xb, hT)
                for m in range(MH):
                    ps = ps1.tile([P, FD], F32)
                    for k in range(KD):
                        nc.tensor.matmul(
                            ps[:, :],
                            w1_sb[:, m, k * P:(k + 1) * P],
                            xb[:, k, :],
                            start=(k == 0),
                            stop=(k == KD - 1),
                        )
                    nc.scalar.activation(
                        hT[:, m, :],
                        ps[:, :],
                        mybir.ActivationFunctionType.Relu,
                        bias=b1_sb[:, m:m + 1] if with_b1 else 0.0,
                    )

            def stage2(blki):
                kind, c0 = blocks[blki]
                last = blki == nblk - 1
                _, hT = live.pop(blki)
                yb = ypool.tile([P, ND, FD], BF16)
                for d in range(ND):
                    if last and d == ND - 1 and kind == "bf":
                        # split the final d over column halves so the very
                        # last DMA (and the end barrier behind it) only
                        # covers 64KB
                        for half in range(2):
                            cc = half * (FD // 2)
                            ps = ps2.tile([P, FD // 2], F32)
                            if kind == "f8":
                                for mp in range(MH // 2):
                                    nc.tensor.matmul(
                                        ps[:, :],
                                        w2q_sb[:, d, 2 * mp:2 * mp + 2, :],
                                        hT[:, 2 * mp:2 * mp + 2, cc:cc + FD // 2],
                                        start=(mp == 0),
                                        stop=(mp == MH // 2 - 1),
                                        perf_mode=mybir.MatmulPerfMode.DoubleRow,
                                    )
                            else:
                                for m in range(MH):
                                    nc.tensor.matmul(
                                        ps[:, :],
                                        w2_sb[:, d, m * P:(m + 1) * P],
                                        hT[:, m, cc:cc + FD // 2],
                                        start=(m == 0),
                                        stop=(m == MH - 1),
                                    )
                            nc.vector.tensor_copy(
                                yb[:, d, cc:cc + FD // 2], ps[:, :]
                            )
                            ring = nc.sync if half == 0 else nc.scalar
                            ring.dma_start(
                                yT_r[:, d, c0 + cc:c0 + cc + FD // 2],
                                yb[:, d, cc:cc + FD // 2],
                            )
                        continue
                    ps = ps2.tile([P, FD], F32)
                    if kind == "f8":
                        for mp in range(MH // 2):
                            nc.tensor.matmul(
                                ps[:, :],
                                w2q_sb[:, d, 2 * mp:2 * mp + 2, :],
                                hT[:, 2 * mp:2 * mp + 2, :],
                                start=(mp == 0),
                                stop=(mp == MH // 2 - 1),
                                perf_mode=mybir.MatmulPerfMode.DoubleRow,
                            )
                    else:
                        for m in range(MH):
                            nc.tensor.matmul(
                                ps[:, :],
                                w2_sb[:, d, m * P:(m + 1) * P],
                                hT[:, m, :],
                                start=(m == 0),
                                stop=(m == MH - 1),
                            )
                    nc.vector.tensor_copy(yb[:, d, :], ps[:, :])
                    if last:
                        # stream the tail out per-d so the end barrier only
                        # waits for one small 128KB transfer
                        ring = nc.sync if d % 2 == 0 else nc.scalar
                        ring.dma_start(yT_r[:, d, c0:c0 + FD], yb[:, d, :])
                if not last:
                    nc.scalar.dma_start(yT_r[:, :, c0:c0 + FD], yb[:, :, :])

            if with_b1 and n_f8:
                b1q_sb = wpool.tile([P, MH], F32)
                nc.vector.tensor_scalar_mul(b1q_sb[:], b1_sb[:], S_H)

            # software-pipelined: stage 1 of block b+1 runs (on PE) between
            # stage 1 and stage 2 of block b, hiding the relu-eviction tail
            stage1(0)
            for b in range(nblk):
                if b + 1 < nblk:
                    stage1(b + 1)
                stage2(b)

    nc.compile()
    return nc


def _route(x_flat, router_w, router_b):
    """Replicates the reference router in numpy float32."""
    logits = x_flat @ router_w + router_b            # [N, E]
    m = logits.max(axis=-1, keepdims=True)
    p = np.exp(logits - m, dtype=np.float32)
    p /= p.sum(axis=-1, keepdims=True)
    # top-k, ties -> lower index first (matches jax.lax.top_k)
    top_i = np.argsort(-p, axis=-1, kind="stable")[:, :TOPK]
    top_p = np.take_along_axis(p, top_i, axis=-1)
    top_p = top_p / top_p.sum(axis=-1, keepdims=True)
    return top_p, top_i


def _f8(a):
    return np.clip(a, -F8_CLIP, F8_CLIP).astype(NP_F8)


def kernel(x, router_w, router_b, w1, b1, w2, b2, _trace=False):
    global last_run_results
    x = np.asarray(x, dtype=np.float32)
    router_w = np.asarray(router_w, dtype=np.float32)
    router_b = np.asarray(router_b, dtype=np.float32)
    w1 = np.asarray(w1, dtype=np.float32)
    b1 = np.asarray(b1, dtype=np.float32)
    w2 = np.asarray(w2, dtype=np.float32)
    b2 = np.asarray(b2, dtype=np.float32)

    N = B * S
    x_flat = x.reshape(N, D)
    top_p, top_i = _route(x_flat, router_w, router_b)

    # Tokens per expert (the "all-to-all by expert assignment"), sorted by
    # gate weight descending: big gates -> bf16 blocks, small gates -> the
    # fp8 block, overflow beyond capacity -> exact host compute.
    idx = []
    gates = []
    for e in range(E):
        ie = np.nonzero((top_i == e).any(axis=-1))[0]
        ge = (top_p[ie] * (top_i[ie] == e)).sum(axis=-1)
        order = np.argsort(-ge, kind="stable")
        idx.append(ie[order])
        gates.append(ge[order])
    counts = np.array([len(i) for i in idx])

    # Device capacity: the perfect-balance point (N*K/8 = 4096).
    n_f8 = N_F8_BLOCKS
    C = max(FD, int(-(-(N * TOPK // N_CORES) // FD) * FD))
    C_bf = C - n_f8 * FD

    with_b1 = bool(np.any(b1))
    key = (C_bf, n_f8, with_b1)
    if key not in _cache:
        _cache[key] = _build(C_bf, n_f8=n_f8, with_b1=with_b1)
    nc = _cache[key]

    in_maps = []
    for e in range(E):
        n_e = min(int(counts[e]), C)
        n_bf = min(n_e, C_bf)
        xTe = np.zeros((D, C_bf), dtype=NP_BF16)
        xTe[:, :n_bf] = x_flat[idx[e][:n_bf]].T
        w1m = np.ascontiguousarray(
            w1[e].reshape(KD, P, MH, P).transpose(2, 1, 0, 3)
        ).astype(NP_BF16)
        w2d = np.ascontiguousarray(
            w2[e].reshape(MH, P, ND, P).transpose(2, 1, 0, 3)
        ).astype(NP_BF16)
        im = {
            "xT": xTe,
            "w1m": w1m,
            "w2d": w2d,
        }
        if n_f8:
            xQe = np.zeros((D, C - C_bf), dtype=NP_F8)
            if n_e > C_bf:
                xQe[:, :n_e - C_bf] = _f8(x_flat[idx[e][C_bf:n_e]].T * S_X)
            im["xq"] = xQe
            im["w1q"] = _f8(
                (w1[e] * S_W1).reshape(KD, P, MH, P).transpose(2, 1, 0, 3)
            )
            im["w2q"] = _f8(
                (w2[e] * S_W2).reshape(MH, P, ND, P).transpose(2, 1, 0, 3)
            )
        if with_b1:
            im["b1t"] = np.ascontiguousarray(b1[e].reshape(MH, P).T)
        in_maps.append(im)

    res = None
    for attempt in range(3):
        try:
            res = bass_utils.run_bass_kernel_spmd(
                nc, in_maps, list(range(N_CORES)), trace=_trace
            )
            break
        except Exception:
            if attempt == 2:
                raise
    last_run_results = res

    # fp8 tokens' outputs come back scaled by S_H*S_W2; fold the descale
    # into their gates.
    out_flat = np.zeros((N, D), dtype=np.float32)
    for e in range(E):
        n_e = min(int(counts[e]), C)
        g = gates[e][:n_e].copy()
        if n_f8 and n_e > C_bf:
            g[C_bf:] /= S_H * S_W2
        y_e = res.results[e]["yT"][:, :n_e].T.astype(np.float32)  # [n_e, D]
        out_flat[idx[e][:n_e]] += g[:, None] * (y_e + b2[e])
        if counts[e] > C:  # host handles the (lowest-gate) overflow tokens
            hi = idx[e][C:]
            h = np.maximum(x_flat[hi] @ w1[e] + b1[e], 0.0)
            y = h @ w2[e] + b2[e]
            out_flat[hi] += gates[e][C:, None] * y
    return out_flat.reshape(B, S, D)
